# revision 7
# baseline (speedup 1.0000x reference)
"""DigitCaps (capsule routing) Trainium2 Bass kernel, v2.

u [512, 1152, 8] f32, W [1, 1152, 10, 16, 8] f32 -> v [512, 10, 16] f32
(3 dynamic-routing iterations, softmax over 10 classes).

Pure data-parallel: batch 64 per core x 8 cores; everything on-chip;
u_hat (377MB) never materialized. Per routing iteration:
  T[b,i,c,k] = sum_d W[i,c,d,k] v[b,c,d]     PE -> PSUM
  evac to bf16 (ACT) or fused mul (DVE-from-PSUM), P = T*u
  Linc[b,i,c] = sum_k P                      PE eye-matmul accumulate
  cE = exp(Linc) [* cE_prev]                 ACT exp from PSUM (+DVE mul)
  den folds on GPSIMD; recip DVE
  xc_c = (u*recT) * cE_c                     DVE / GPSIMD split
  s[b,c,:] = sum_{ik} W xc_c                 PE accumulating matmuls
  v = squash(s)
exp(L1+L2) == exp(L1)*exp(L2), so logits are never materialized.

Layouts (per core, B=64):
  i: block g = i//128 (9 blocks), partition r = i%128
  class c = 2p+ch, pass p in [0,5), parity ch in {0,1}
  exp/cE: [r, p, (g, ch, b)]
"""

import os
import numpy as np

N_CORES = 8
B_PER = 64
I_CAPS = 1152
K_DIM = 8
C_CLS = 10
D_DIM = 16
NG = I_CAPS // 128  # 9
EPS = 1e-8

# --- schedule knobs (cost-model balancing) ---
Z_KS = tuple(
    int(x) for x in os.environ.get("KV2_ZKS", "1,3,5,7").split(",") if x != ""
)  # k-indices whose T*u mul reads PSUM directly on DVE (no ACT evac)
POOL_MUL_KS = tuple(
    int(x) for x in os.environ.get("KV2_PMKS", "").split(",") if x != ""
)  # k-indices whose (evac'd) mul runs on GPSIMD
XC_POOL = tuple(
    int(x) for x in os.environ.get("KV2_XCPOOL", "").split(",") if x != ""
)  # classes whose xc mul runs on GPSIMD
XC_DMA0 = tuple(
    int(x) for x in os.environ.get("KV2_XCDMA0", "").split(",") if x != ""
)  # round-1 classes whose xc mul runs as SP-copy + gpsimd DMA-accum-mult
XC_DMA1 = tuple(
    int(x) for x in os.environ.get("KV2_XCDMA1", "").split(",") if x != ""
)  # round-2 classes on the DMA-mult route
XC_GPOOL = int(os.environ.get("KV2_XCGPOOL", "2"))  # trailing g-blocks on Pool
FOLDS_POOL = os.environ.get("KV2_FOLDSPOOL", "1") == "1"
CE_POOL = os.environ.get("KV2_CEPOOL", "1") == "1"
CE_ALT = os.environ.get("KV2_CEALT", "1") == "1"
FP8_T = os.environ.get("KV2_FP8", "0") == "1"  # DoubleRow fp8 T matmuls
W8SCALE = 256.0  # exact power-of-two prescale lifting fp8 W out of subnormals

_CACHE = {}


def _build():
    import concourse.bass as bass
    import concourse.mybir as mybir
    from concourse import tile, bacc

    f32 = mybir.dt.float32
    bf16 = mybir.dt.bfloat16
    f8 = mybir.dt.float8e4
    AF = mybir.ActivationFunctionType
    OP = mybir.AluOpType
    PM = mybir.MatmulPerfMode

    nc = bacc.Bacc()
    uTk_in = nc.dram_tensor(
        "uTk_h", [128, K_DIM, NG, B_PER], bf16, kind="ExternalInput"
    )
    wsk_in = nc.dram_tensor(
        "wsk_h", [128, K_DIM, NG, C_CLS, D_DIM], bf16, kind="ExternalInput"
    )
    # wt cols 0:1152 = rows 16c+d classes 0-7 (all 128 partitions);
    # cols 1152:2304 = rows 16(c-8)+d classes 8,9 (partitions 0:32)
    wdt = f8 if FP8_T else bf16
    if FP8_T:
        # DoubleRow layout: [32p+8cc+d//2, k, d%2, i(+1152 for classes 8,9)]
        wt_in = nc.dram_tensor(
            "wt_h", [128, K_DIM, 2, 2 * I_CAPS], f8, kind="ExternalInput"
        )
    else:
        wt_in = nc.dram_tensor(
            "wt_h", [128, K_DIM, 2 * I_CAPS], bf16, kind="ExternalInput"
        )
    eyebf = nc.dram_tensor("eyebf", [128, 128], bf16, kind="ExternalInput")
    v_out = nc.dram_tensor("v", [B_PER, C_CLS, D_DIM], f32, kind="ExternalOutput")
    vdr = nc.dram_tensor("vdr", [2, 5, B_PER, 2, D_DIM], wdt, kind="Internal")

    with tile.TileContext(nc) as tc:
        perm = tc.alloc_tile_pool(name="perm", bufs=1)
        Wsk = perm.tile([128, K_DIM, NG, C_CLS, D_DIM], bf16)  # [r,(k,g,c,d)]
        if FP8_T:
            WT = perm.tile([128, K_DIM, 2, 2 * I_CAPS], f8)
        else:
            WT = perm.tile([128, K_DIM, 2 * I_CAPS], bf16)
        uTk = perm.tile([128, K_DIM, NG, B_PER], bf16)      # u[b, 128g+r, k]
        # exp(L); layout [r, p, ch, g, b] so per-(p,ch) slices are contiguous
        cEa = perm.tile([128, 5, 2, NG, B_PER], bf16, name="cEa")
        cEb = perm.tile([128, 5, 2, NG, B_PER], bf16, name="cEb")
        recT = perm.tile([128, NG, B_PER], bf16, name="recTt")    # 1/den i-major
        if FP8_T:
            # DoubleRow rhs: [32p+8cc+d//2, d%2, 64cc+b]; pass 4 in vT4 rows 0:16
            vT = perm.tile([128, 2, 128], f8)
            vT4 = perm.tile([128, 2, 128], f8)
        else:
            vT = perm.tile([128, 128], bf16)         # block-diag v^T classes 0-7
            vT4 = perm.tile([128, 128], bf16)        # rows 0:32: classes 8,9
        v_sb = perm.tile([64, C_CLS, D_DIM], f32, name="vsbt")
        vbf = perm.tile([64, C_CLS, D_DIM], wdt, name="vbft")
        s_sb = perm.tile([64, C_CLS, D_DIM], f32, name="ssbt")
        eyeb_sb = perm.tile([128, 128], bf16)
        den = perm.tile([128, NG, B_PER], bf16, name="dent")
        dtmp = perm.tile([128, NG, B_PER], bf16, name="dtmpt")

        psT = tc.alloc_tile_pool(name="psT", bufs=2, space="PSUM")
        psL = tc.alloc_tile_pool(name="psL", bufs=2, space="PSUM")

        # ---------------- setup: inputs arrive pre-arranged ----
        for k0 in range(0, K_DIM, 2):
            nc.sync.dma_start(uTk[:, k0 : k0 + 2], uTk_in[:, k0 : k0 + 2])
            nc.sync.dma_start(Wsk[:, k0 : k0 + 2], wsk_in[:, k0 : k0 + 2])
        nc.sync.dma_start(eyeb_sb[:], eyebf[:])
        if FP8_T:
            for p5 in range(4):
                nc.sync.dma_start(
                    WT[32 * p5 : 32 * p5 + 16, :, :, 0:I_CAPS],
                    wt_in[32 * p5 : 32 * p5 + 16, :, :, 0:I_CAPS],
                )
            nc.sync.dma_start(WT[0:16, :, :, I_CAPS:], wt_in[0:16, :, :, I_CAPS:])
        else:
            nc.sync.dma_start(WT[:, 0:4, 0:I_CAPS], wt_in[:, 0:4, 0:I_CAPS])
            nc.sync.dma_start(WT[0:32, 0:4, I_CAPS:], wt_in[0:32, 0:4, I_CAPS:])
            nc.sync.dma_start(WT[:, 4:8, 0:I_CAPS], wt_in[:, 4:8, 0:I_CAPS])
            nc.sync.dma_start(WT[0:32, 4:8, I_CAPS:], wt_in[0:32, 4:8, I_CAPS:])

        nc.gpsimd.memset(vT[:], 0.0)
        nc.gpsimd.memset(vT4[:], 0.0)

        # PE warmup: keep the tensor engine continuously busy through the
        # input-DMA window so s0 and round 1 run at the ramped clock.
        wu = perm.tile([128, 512], bf16, name="wut")
        nc.vector.memset(wu[:], 0.0)

        itp = tc.alloc_tile_pool(name="itp", bufs=2)
        smp = tc.alloc_tile_pool(name="smp", bufs=3)

        def pe_warmup(n):
            for _ in range(n):
                wt = psT.tile([128, I_CAPS], f32, tag="pt")
                nc.tensor.matmul(
                    wt[:, 0:512], eyeb_sb[:], wu[:], start=True, stop=True
                )

        def s_phase_s0_pair(pp):
            """Uniform-c s for classes 2pp,2pp+1 only, so the first routing
            round can start on a class pair before s0 fully finishes."""
            pst = psL.tile([128, 512], f32, tag="lp")
            ps = pst[0:64, 0 : 2 * D_DIM]
            n = 0
            for k in range(K_DIM):
                for g in range(NG):
                    nc.tensor.matmul(
                        ps,
                        uTk[:, k, g, :],
                        Wsk[:, k, g, 2 * pp : 2 * pp + 2, :].rearrange(
                            "r c d -> r (c d)"
                        ),
                        start=(n == 0),
                        stop=(n == K_DIM * NG - 1),
                    )
                    n += 1
            nc.scalar.activation(
                s_sb[:, 2 * pp : 2 * pp + 2, :].rearrange("b c d -> b (c d)"),
                ps,
                AF.Copy,
                scale=0.1,
            )

        def squash_pair(p, final=False):
            """squash for classes 2p, 2p+1 only; writes bf16 vbf slices
            (or f32 v_sb when final).
            fac = n2 / ((1 + n2) * (sqrt(n2) + EPS)), v = fac * s."""
            sqp = smp.tile([64, 2, D_DIM], f32, tag="sqp", bufs=2)
            n2p = smp.tile([64, 2], f32, tag="n2p", bufs=2)
            nrp = smp.tile([64, 2], f32, tag="nrp", bufs=2)
            dnp = smp.tile([64, 2], f32, tag="dnp", bufs=2)
            rcp = smp.tile([64, 2], f32, tag="rcp", bufs=2)
            fcp = smp.tile([64, 2], f32, tag="fcp", bufs=2)
            sv = s_sb[:, 2 * p : 2 * p + 2, :]
            nc.scalar.square(sqp[:], sv)
            nc.vector.reduce_sum(n2p[:], sqp[:], axis=mybir.AxisListType.X)
            nc.scalar.sqrt(nrp[:], n2p[:])
            nc.vector.tensor_scalar_add(nrp[:], nrp[:], EPS)
            # dnp = (n2p + 1) * nrp
            nc.vector.scalar_tensor_tensor(
                dnp[:], n2p[:], 1.0, nrp[:], OP.add, OP.mult
            )
            nc.vector.reciprocal(rcp[:], dnp[:])
            nc.vector.tensor_mul(fcp[:], n2p[:], rcp[:])
            if final:
                for cc in range(2):
                    c = 2 * p + cc
                    nc.vector.tensor_scalar_mul(
                        v_sb[:, c, :], sv[:, cc, :], fcp[:, cc : cc + 1]
                    )
            else:
                with nc.allow_low_precision(reason="v to bf16 for T matmuls"):
                    for cc in range(2):
                        c = 2 * p + cc
                        nc.vector.tensor_scalar_mul(
                            vbf[:, c, :], sv[:, cc, :], fcp[:, cc : cc + 1]
                        )

        def vT_write_pair(p, slot):
            """Stage classes 2p,2p+1 of vbf in DRAM (SP-initiated)."""
            nc.sync.dma_start(vdr[slot, p], vbf[:, 2 * p : 2 * p + 2, :])

        def vT_read_pair(p, slot):
            """Read a staged pair back transposed into its block-diagonal vT
            slots. bf16: vT[32p+16cc+d, 64cc+b] = v[b,2p+cc,d]. fp8/DoubleRow:
            vT[32p+8cc+d//2, d%2, 64cc+b]. Reads spread over SP/ACT queues."""
            dst_tile = vT if p < 4 else vT4
            r0 = 32 * p if p < 4 else 0
            if FP8_T:
                engs = (nc.sync, nc.scalar)
                n = 0
                for cc in range(2):
                    for ko in range(2):
                        engs[n % 2].dma_start(
                            dst_tile[r0 + 8 * cc : r0 + 8 * cc + 8, ko,
                                     64 * cc : 64 * cc + 64],
                            vdr[slot, p, :, cc, ko::2].rearrange("b d -> d b"),
                        )
                        n += 1
            else:
                for eng, cc in ((nc.sync, 0), (nc.scalar, 1)):
                    eng.dma_start(
                        dst_tile[r0 + 16 * cc : r0 + 16 * cc + 16,
                                 64 * cc : 64 * cc + 64],
                        vdr[slot, p, :, cc, :].rearrange("b d -> d b"),
                    )

        def vT_dma_pair(p, slot):
            vT_write_pair(p, slot)
            vT_read_pair(p, slot)

        def L_front(j, p):
            """T matmuls + evac/mul for pass p; returns the P tile."""
            if p < 4:
                row0, col0 = 32 * p, 0
            else:
                row0, col0 = 0, I_CAPS
            if FP8_T:
                vrhs = (vT if p < 4 else vT4)[row0 : row0 + 16, :, :]
            else:
                vrhs = (vT if p < 4 else vT4)[row0 : row0 + 32, :]
            P = itp.tile([128, K_DIM, NG, 128], bf16, tag="pp")
            for k in range(K_DIM):
                pt = psT.tile([128, I_CAPS], f32, tag="pt")
                for g in range(NG):
                    if FP8_T:
                        nc.tensor.matmul(
                            pt[:, 128 * g : 128 * (g + 1)],
                            WT[row0 : row0 + 16, k, :,
                               col0 + 128 * g : col0 + 128 * (g + 1)],
                            vrhs,
                            start=True,
                            stop=True,
                            tile_position=(row0, 0),
                            perf_mode=PM.DoubleRow,
                        )
                    else:
                        nc.tensor.matmul(
                            pt[:, 128 * g : 128 * (g + 1)],
                            WT[row0 : row0 + 32, k,
                               col0 + 128 * g : col0 + 128 * (g + 1)],
                            vrhs,
                            start=True,
                            stop=True,
                            tile_position=(row0, 0),
                        )
                ubc = uTk[:, k].rearrange("r g b -> r g () b").to_broadcast(
                    (128, NG, 2, B_PER)
                )
                pk = P[:, k].rearrange("r g (c b) -> r g c b", c=2)
                if k in Z_KS:
                    # fused: P_k = T_k(PSUM) * u_k on DVE
                    nc.vector.tensor_tensor(
                        pk,
                        pt[:].rearrange("r (g c b) -> r g c b", g=NG, c=2),
                        ubc,
                        OP.mult,
                    )
                else:
                    Tp = itp.tile([128, NG, 128], bf16, tag="tp", bufs=6)
                    nc.scalar.copy(Tp[:].rearrange("r g q -> r (g q)"), pt[:])
                    tv = Tp[:].rearrange("r g (c b) -> r g c b", c=2)
                    # scalar_tensor_tensor (TensorScalarPtr) runs in the DVE
                    # 4x_2p perf mode on all-SBUF packed bf16 operands; plain
                    # tensor_tensor only reaches 2x.
                    if k in POOL_MUL_KS:
                        nc.gpsimd.scalar_tensor_tensor(
                            pk, tv, 1.0, ubc, OP.mult, OP.mult
                        )
                    else:
                        nc.vector.scalar_tensor_tensor(
                            pk, tv, 1.0, ubc, OP.mult, OP.mult
                        )
            return P

        def L_back(j, p, P, bts, cE_prev, cE):
            """ksum + exp + cE product + denominator folds for pass p."""
            Pf = P[:].rearrange("r k g q -> r k (g q)")

            def cegv(t, g0, g1):
                # [r, g, ch, b] view over g block (enumeration order of Linc)
                return t[:, p, :, g0:g1, :].rearrange("r c g b -> r g c b")

            for g0, g1 in ((0, 4), (4, 8), (8, NG)):
                s, e = 128 * g0, 128 * g1
                lp = psL.tile([128, 512], f32, tag="lp")
                for k in range(K_DIM):
                    nc.tensor.matmul(
                        lp[:, 0 : e - s],
                        eyeb_sb[:],
                        Pf[:, k, s:e],
                        start=(k == 0),
                        stop=(k == K_DIM - 1),
                    )
                lpv = lp[:, 0 : e - s].rearrange(
                    "r (g c b) -> r g c b", c=2, b=B_PER
                )
                if j == 0:
                    nc.scalar.activation(
                        cegv(cE, g0, g1), lpv, AF.Exp,
                        scale=(1.0 / W8SCALE if FP8_T else 1.0),
                    )
                else:
                    Er = itp.tile([128, 512], bf16, tag="er", bufs=3)
                    erv = Er[:, 0 : e - s].rearrange(
                        "r (g c b) -> r g c b", c=2, b=B_PER
                    )
                    nc.scalar.activation(
                        erv, lpv, AF.Exp,
                        scale=(1.0 / W8SCALE if FP8_T else 1.0),
                    )
                    ce_eng = (
                        nc.gpsimd
                        if (CE_POOL and (g0 < 8 or not CE_ALT))
                        else nc.vector
                    )
                    ce_eng.scalar_tensor_tensor(
                        cegv(cE, g0, g1), erv, 1.0, cegv(cE_prev, g0, g1),
                        OP.mult, OP.mult,
                    )
            # progressive denominator folds; the last pass's fold and the
            # final partial sum sit on the recT critical chain, so they go
            # to DVE even when the earlier folds run on GPSIMD
            eng = nc.gpsimd if (FOLDS_POOL and p < 3) else nc.vector
            eng.scalar_tensor_tensor(
                bts[p][:], cE[:, p, 0], 1.0, cE[:, p, 1], OP.mult, OP.add
            )
            if p == 1:
                eng.scalar_tensor_tensor(
                    bts[0][:], bts[0][:], 1.0, bts[1][:], OP.mult, OP.add
                )
            elif p == 3:
                eng.scalar_tensor_tensor(
                    bts[2][:], bts[2][:], 1.0, bts[3][:], OP.mult, OP.add
                )
                # dtmp = (b0+b1) + (b2+b3) ready before pass 4 lands
                nc.vector.scalar_tensor_tensor(
                    dtmp[:], bts[0][:], 1.0, bts[2][:], OP.mult, OP.add
                )

        def L_phase(j, bts):
            """Software-pipelined: pass p's back-end is emitted after pass
            p+1's front-end so its cross-engine waits are already satisfied
            when they reach the strict-FIFO engine queues."""
            cE_prev, cE = (None, cEa) if j == 0 else (cEa, cEb)
            Ps = [None] * 5
            for p in range(5):
                Ps[p] = L_front(j, p)
                if p >= 1:
                    L_back(j, p - 1, Ps[p - 1], bts, cE_prev, cE)
            L_back(j, 4, Ps[4], bts, cE_prev, cE)
            return cE

        def softmax_phase(bts):
            nc.vector.scalar_tensor_tensor(
                den[:], dtmp[:], 1.0, bts[4][:], OP.mult, OP.add
            )
            with nc.allow_low_precision(reason="softmax reciprocal to bf16 ok"):
                nc.vector.reciprocal(
                    recT[:].rearrange("r g b -> r (g b)"),
                    den[:].rearrange("r g b -> r (g b)"),
                )

        def s_phase_routed(cE, pipe_vT, xc_dma):
            uTs = itp.tile([128, K_DIM, NG, B_PER], bf16, tag="uts", bufs=1)
            nc.vector.scalar_tensor_tensor(
                uTs[:],
                uTk[:],
                1.0,
                recT[:].rearrange("r g b -> r () g b").to_broadcast(
                    (128, K_DIM, NG, B_PER)
                ),
                OP.mult,
                OP.mult,
            )
            def xc_front(c):
                p, ch = c // 2, c % 2
                xc = itp.tile(
                    [128, K_DIM, NG, B_PER], bf16,
                    tag=("xcd" if c in xc_dma else "xc"),
                    bufs=(1 if c in xc_dma else 2),
                )
                cbc = cE[:, p, ch].rearrange("r g b -> r () g b").to_broadcast(
                    (128, K_DIM, NG, B_PER)
                )
                if c in xc_dma:
                    # xc = us (SP DMA copy), then xc *= cE_c (gpsimd DMA
                    # with CCE multiply; src broadcast over k)
                    nc.sync.dma_start(xc[:], uTs[:])
                    nc.gpsimd.dma_start(xc[:], cbc, accum_op=OP.mult)
                elif c in XC_POOL:
                    nc.gpsimd.scalar_tensor_tensor(
                        xc[:], uTs[:], 1.0, cbc, OP.mult, OP.mult
                    )
                elif XC_GPOOL > 0:
                    gs = NG - XC_GPOOL
                    cb = cE[:, p, ch].rearrange("r g b -> r () g b")
                    nc.vector.scalar_tensor_tensor(
                        xc[:, :, 0:gs],
                        uTs[:, :, 0:gs],
                        1.0,
                        cb[:, :, 0:gs].to_broadcast((128, K_DIM, gs, B_PER)),
                        OP.mult,
                        OP.mult,
                    )
                    nc.gpsimd.scalar_tensor_tensor(
                        xc[:, :, gs:],
                        uTs[:, :, gs:],
                        1.0,
                        cb[:, :, gs:].to_broadcast(
                            (128, K_DIM, XC_GPOOL, B_PER)
                        ),
                        OP.mult,
                        OP.mult,
                    )
                else:
                    nc.vector.scalar_tensor_tensor(
                        xc[:], uTs[:], 1.0, cbc, OP.mult, OP.mult
                    )
                return xc

            def s_back(c, xc):
                pst = psL.tile([128, 512], f32, tag="lp")
                ps = pst[0:64, 0:D_DIM]
                n = 0
                for k in range(K_DIM):
                    for g in range(NG):
                        nc.tensor.matmul(
                            ps,
                            xc[:, k, g, :],
                            Wsk[:, k, g, c, :],
                            start=(n == 0),
                            stop=(n == K_DIM * NG - 1),
                        )
                        n += 1
                nc.scalar.copy(s_sb[:, c, :], ps)

            def pair_done(cdone):
                """Classes 2p,2p+1 are in s_sb: squash the pair; round 1
                DMAs the block into vT so the next round's T matmuls can
                start, round 2 DMAs the f32 result out early."""
                if cdone % 2 != 1:
                    return
                p = cdone // 2
                if pipe_vT:
                    squash_pair(p)
                    vT_dma_pair(p, 1)
                else:
                    squash_pair(p, final=True)
                    nc.sync.dma_start(
                        v_out[:, 2 * p : 2 * p + 2, :], v_sb[:, 2 * p : 2 * p + 2, :]
                    )

            xcs = [None] * C_CLS
            for c in range(C_CLS):
                xcs[c] = xc_front(c)
                if c >= 1:
                    s_back(c - 1, xcs[c - 1])
                    pair_done(c - 1)
            s_back(C_CLS - 1, xcs[C_CLS - 1])
            pair_done(C_CLS - 1)

        # ---------------- main flow ----------------
        kstage = int(os.environ.get("KSTAGE", "99"))
        wu_n = int(os.environ.get("KV2_WU", "8"))
        for p in range(5):
            s_phase_s0_pair(p)
            if p == 0 and wu_n:
                # low-priority gap fillers: run only when the PE would
                # otherwise idle waiting on the input DMA stream
                pe_warmup(wu_n)
            squash_pair(p)
            vT_dma_pair(p, 0)
        if kstage >= 1:
            for j in range(2):
                bts = []
                for i in range(5):
                    bti = smp.tile(
                        [128, NG, B_PER], bf16, tag=f"sm{i}", bufs=2,
                        name=f"bt{i}_{j}",
                    )
                    bts.append(bti)
                cE = L_phase(j, bts)
                if kstage == 1 + 3 * j:
                    break
                softmax_phase(bts)
                if kstage == 2 + 3 * j:
                    break
                s_phase_routed(
                    cE, pipe_vT=(j == 0), xc_dma=(XC_DMA0 if j == 0 else XC_DMA1)
                )
                if kstage == 3 + 3 * j:
                    break

        for pool in (smp, itp, psL, psT, perm):
            try:
                pool.release()
            except Exception:
                pass

    nc.compile()
    return nc


def _consts():
    import ml_dtypes

    return {"eyebf": np.eye(128, dtype=np.float32).astype(ml_dtypes.bfloat16)}


def _prep_w(W0):
    """Host-side layout marshalling of the replicated weights (pure
    permutation + bf16 cast; done once, shared by all cores)."""
    import ml_dtypes

    bf = ml_dtypes.bfloat16
    W0 = np.ascontiguousarray(W0, dtype=np.float32)  # [1152, 10, 16, 8]
    wsk = np.ascontiguousarray(
        W0.reshape(NG, 128, C_CLS, D_DIM, K_DIM).transpose(1, 4, 0, 2, 3)
    ).astype(bf)  # [128, k, g, c, d]
    if FP8_T:
        f8 = ml_dtypes.float8_e4m3
        wt = np.zeros((128, K_DIM, 2, 2 * I_CAPS), dtype=f8)
        Ws = (W0 * W8SCALE).astype(f8)
        for p in range(5):
            r0, c0 = (32 * p, 0) if p < 4 else (0, I_CAPS)
            for cc in range(2):
                for d in range(D_DIM):
                    # [i, k] -> row r0+8cc+d//2, ko=d%2
                    wt[r0 + 8 * cc + d // 2, :, d % 2, c0 : c0 + I_CAPS] = Ws[
                        :, 2 * p + cc, d, :
                    ].T
        return wsk, wt
    wt = np.zeros((128, K_DIM, 2 * I_CAPS), dtype=bf)
    wt[:, :, 0:I_CAPS] = (
        W0[:, 0:8].transpose(1, 2, 3, 0).reshape(128, K_DIM, I_CAPS).astype(bf)
    )  # rows 16c+d, classes 0-7
    wt[0:32, :, I_CAPS:] = (
        W0[:, 8:10].transpose(1, 2, 3, 0).reshape(32, K_DIM, I_CAPS).astype(bf)
    )  # rows 16(c-8)+d, classes 8,9
    return wsk, wt


def _prep_u(ush):
    import ml_dtypes

    return np.ascontiguousarray(
        ush.reshape(B_PER, NG, 128, K_DIM).transpose(2, 3, 1, 0)
    ).astype(ml_dtypes.bfloat16)  # [128, k, g, b]


def get_nc():
    if "nc" not in _CACHE:
        _CACHE["nc"] = _build()
    return _CACHE["nc"]


def make_in_maps(u, W):
    consts = _consts()
    wsk, wt = _prep_w(W[0])
    in_maps = []
    for core in range(N_CORES):
        sh = np.ascontiguousarray(
            u[core * B_PER : (core + 1) * B_PER], dtype=np.float32
        )
        in_maps.append(
            {
                "uTk_h": _prep_u(sh),
                "wsk_h": wsk,
                "wt_h": wt,
                **consts,
            }
        )
    return in_maps


def kernel(u: np.ndarray, W: np.ndarray) -> np.ndarray:
    from concourse.bass_utils import run_bass_kernel_spmd

    nc = get_nc()
    in_maps = make_in_maps(u, W)
    res = run_bass_kernel_spmd(nc, in_maps, list(range(N_CORES)))
    out = np.concatenate([res.results[i]["v"] for i in range(N_CORES)], axis=0)
    return out.astype(np.float32)



# revision 9
# speedup vs baseline: 1.2891x; 1.2891x over previous
"""DigitCaps (capsule routing) Trainium2 Bass kernel, v2.

u [512, 1152, 8] f32, W [1, 1152, 10, 16, 8] f32 -> v [512, 10, 16] f32
(3 dynamic-routing iterations, softmax over 10 classes).

Pure data-parallel: batch 64 per core x 8 cores; everything on-chip;
u_hat (377MB) never materialized. Per routing iteration:
  T[b,i,c,k] = sum_d W[i,c,d,k] v[b,c,d]     PE -> PSUM
  evac to bf16 (ACT) or fused mul (DVE-from-PSUM), P = T*u
  Linc[b,i,c] = sum_k P                      PE eye-matmul accumulate
  cE = exp(Linc) [* cE_prev]                 ACT exp from PSUM (+DVE mul)
  den folds on GPSIMD; recip DVE
  xc_c = (u*recT) * cE_c                     DVE / GPSIMD split
  s[b,c,:] = sum_{ik} W xc_c                 PE accumulating matmuls
  v = squash(s)
exp(L1+L2) == exp(L1)*exp(L2), so logits are never materialized.

Layouts (per core, B=64):
  i: block g = i//128 (9 blocks), partition r = i%128
  class c = 2p+ch, pass p in [0,5), parity ch in {0,1}
  exp/cE: [r, p, (g, ch, b)]
"""

import os
import numpy as np

N_CORES = 8
B_PER = 64
I_CAPS = 1152
K_DIM = 8
C_CLS = 10
D_DIM = 16
NG = I_CAPS // 128  # 9
EPS = 1e-8

# --- schedule knobs (cost-model balancing) ---
Z_KS = tuple(
    int(x) for x in os.environ.get("KV2_ZKS", "1,5").split(",") if x != ""
)  # k-indices whose T*u mul reads PSUM directly on DVE (no ACT evac)
POOL_MUL_KS = tuple(
    int(x) for x in os.environ.get("KV2_PMKS", "0,2,4,6").split(",") if x != ""
)  # k-indices whose (evac'd) mul runs on GPSIMD
XC_POOL = tuple(
    int(x) for x in os.environ.get("KV2_XCPOOL", "").split(",") if x != ""
)  # classes whose xc mul runs on GPSIMD
XC_DMA0 = tuple(
    int(x) for x in os.environ.get("KV2_XCDMA0", "").split(",") if x != ""
)  # round-1 classes whose xc mul runs as SP-copy + gpsimd DMA-accum-mult
XC_DMA1 = tuple(
    int(x) for x in os.environ.get("KV2_XCDMA1", "").split(",") if x != ""
)  # round-2 classes on the DMA-mult route
XC_GPOOL = int(os.environ.get("KV2_XCGPOOL", "1"))  # trailing g-blocks on Pool
FOLDS_POOL = os.environ.get("KV2_FOLDSPOOL", "1") == "1"
CE_POOL = os.environ.get("KV2_CEPOOL", "0") == "1"
CE_ALT = os.environ.get("KV2_CEALT", "1") == "1"
FP8_T = os.environ.get("KV2_FP8", "0") == "1"  # DoubleRow fp8 T matmuls
W8SCALE = 256.0  # exact power-of-two prescale lifting fp8 W out of subnormals

_CACHE = {}


def _build():
    import concourse.bass as bass
    import concourse.mybir as mybir
    from concourse import tile, bacc

    f32 = mybir.dt.float32
    bf16 = mybir.dt.bfloat16
    f8 = mybir.dt.float8e4
    AF = mybir.ActivationFunctionType
    OP = mybir.AluOpType
    PM = mybir.MatmulPerfMode

    nc = bacc.Bacc()
    uTk_in = nc.dram_tensor(
        "uTk_h", [128, K_DIM, NG, B_PER], bf16, kind="ExternalInput"
    )
    wsk_in = nc.dram_tensor(
        "wsk_h", [128, K_DIM, NG, C_CLS, D_DIM], bf16, kind="ExternalInput"
    )
    # wt cols 0:1152 = rows 16c+d classes 0-7 (all 128 partitions);
    # cols 1152:2304 = rows 16(c-8)+d classes 8,9 (partitions 0:32)
    wdt = f8 if FP8_T else bf16
    if FP8_T:
        # DoubleRow layout: [32p+8cc+d//2, k, d%2, i(+1152 for classes 8,9)]
        wt_in = nc.dram_tensor(
            "wt_h", [128, K_DIM, 2, 2 * I_CAPS], f8, kind="ExternalInput"
        )
    else:
        wt_in = nc.dram_tensor(
            "wt_h", [128, K_DIM, 2 * I_CAPS], bf16, kind="ExternalInput"
        )
    eyebf = nc.dram_tensor("eyebf", [128, 128], bf16, kind="ExternalInput")
    v_out = nc.dram_tensor("v", [B_PER, C_CLS, D_DIM], f32, kind="ExternalOutput")
    vdr = nc.dram_tensor("vdr", [2, 5, B_PER, 2, D_DIM], wdt, kind="Internal")

    with tile.TileContext(nc) as tc:
        perm = tc.alloc_tile_pool(name="perm", bufs=1)
        Wsk = perm.tile([128, K_DIM, NG, C_CLS, D_DIM], bf16)  # [r,(k,g,c,d)]
        if FP8_T:
            WT = perm.tile([128, K_DIM, 2, 2 * I_CAPS], f8)
        else:
            WT = perm.tile([128, K_DIM, 2 * I_CAPS], bf16)
        uTk = perm.tile([128, K_DIM, NG, B_PER], bf16)      # u[b, 128g+r, k]
        # exp(L); layout [r, p, ch, g, b] so per-(p,ch) slices are contiguous
        cEa = perm.tile([128, 5, 2, NG, B_PER], bf16, name="cEa")
        cEb = perm.tile([128, 5, 2, NG, B_PER], bf16, name="cEb")
        recT = perm.tile([128, NG, B_PER], bf16, name="recTt")    # 1/den i-major
        if FP8_T:
            # DoubleRow rhs: [32p+8cc+d//2, d%2, 64cc+b]; pass 4 in vT4 rows 0:16
            vT = perm.tile([128, 2, 128], f8)
            vT4 = perm.tile([128, 2, 128], f8)
        else:
            vT = perm.tile([128, 128], bf16)         # block-diag v^T classes 0-7
            vT4 = perm.tile([128, 128], bf16)        # rows 0:32: classes 8,9
        v_sb = perm.tile([64, C_CLS, D_DIM], f32, name="vsbt")
        vbf = perm.tile([64, C_CLS, D_DIM], wdt, name="vbft")
        s_sb = perm.tile([64, C_CLS, D_DIM], f32, name="ssbt")
        eyeb_sb = perm.tile([128, 128], bf16)
        den = perm.tile([128, NG, B_PER], bf16, name="dent")
        dtmp = perm.tile([128, NG, B_PER], bf16, name="dtmpt")

        psT = tc.alloc_tile_pool(name="psT", bufs=2, space="PSUM")
        psL = tc.alloc_tile_pool(name="psL", bufs=2, space="PSUM")

        # ---------------- setup: inputs arrive pre-arranged ----
        for k0 in range(0, K_DIM, 2):
            nc.sync.dma_start(uTk[:, k0 : k0 + 2], uTk_in[:, k0 : k0 + 2])
            nc.sync.dma_start(Wsk[:, k0 : k0 + 2], wsk_in[:, k0 : k0 + 2])
        nc.sync.dma_start(eyeb_sb[:], eyebf[:])
        if FP8_T:
            for p5 in range(4):
                nc.sync.dma_start(
                    WT[32 * p5 : 32 * p5 + 16, :, :, 0:I_CAPS],
                    wt_in[32 * p5 : 32 * p5 + 16, :, :, 0:I_CAPS],
                )
            nc.sync.dma_start(WT[0:16, :, :, I_CAPS:], wt_in[0:16, :, :, I_CAPS:])
        else:
            nc.sync.dma_start(WT[:, 0:4, 0:I_CAPS], wt_in[:, 0:4, 0:I_CAPS])
            nc.sync.dma_start(WT[0:32, 0:4, I_CAPS:], wt_in[0:32, 0:4, I_CAPS:])
            nc.sync.dma_start(WT[:, 4:8, 0:I_CAPS], wt_in[:, 4:8, 0:I_CAPS])
            nc.sync.dma_start(WT[0:32, 4:8, I_CAPS:], wt_in[0:32, 4:8, I_CAPS:])

        nc.gpsimd.memset(vT[:], 0.0)
        nc.gpsimd.memset(vT4[:], 0.0)

        # PE warmup: keep the tensor engine continuously busy through the
        # input-DMA window so s0 and round 1 run at the ramped clock.
        wu = perm.tile([128, 512], bf16, name="wut")
        nc.vector.memset(wu[:], 0.0)

        itp = tc.alloc_tile_pool(name="itp", bufs=2)
        smp = tc.alloc_tile_pool(name="smp", bufs=3)

        def pe_warmup(n):
            for _ in range(n):
                wt = psT.tile([128, I_CAPS], f32, tag="pt")
                nc.tensor.matmul(
                    wt[:, 0:512], eyeb_sb[:], wu[:], start=True, stop=True
                )

        def s_phase_s0_pair(pp):
            """Uniform-c s for classes 2pp,2pp+1 only, so the first routing
            round can start on a class pair before s0 fully finishes."""
            pst = psL.tile([128, 512], f32, tag="lp")
            ps = pst[0:64, 0 : 2 * D_DIM]
            n = 0
            for k in range(K_DIM):
                for g in range(NG):
                    nc.tensor.matmul(
                        ps,
                        uTk[:, k, g, :],
                        Wsk[:, k, g, 2 * pp : 2 * pp + 2, :].rearrange(
                            "r c d -> r (c d)"
                        ),
                        start=(n == 0),
                        stop=(n == K_DIM * NG - 1),
                    )
                    n += 1
            nc.scalar.activation(
                s_sb[:, 2 * pp : 2 * pp + 2, :].rearrange("b c d -> b (c d)"),
                ps,
                AF.Copy,
                scale=0.1,
            )

        def squash_pair(p, final=False):
            """squash for classes 2p, 2p+1 only; writes bf16 vbf slices
            (or f32 v_sb when final).
            fac = n2 / ((1 + n2) * (sqrt(n2) + EPS)), v = fac * s."""
            sqp = smp.tile([64, 2, D_DIM], f32, tag="sqp", bufs=2)
            n2p = smp.tile([64, 2], f32, tag="n2p", bufs=2)
            nrp = smp.tile([64, 2], f32, tag="nrp", bufs=2)
            dnp = smp.tile([64, 2], f32, tag="dnp", bufs=2)
            rcp = smp.tile([64, 2], f32, tag="rcp", bufs=2)
            fcp = smp.tile([64, 2], f32, tag="fcp", bufs=2)
            sv = s_sb[:, 2 * p : 2 * p + 2, :]
            nc.scalar.square(sqp[:], sv)
            nc.vector.reduce_sum(n2p[:], sqp[:], axis=mybir.AxisListType.X)
            nc.scalar.sqrt(nrp[:], n2p[:])
            nc.vector.tensor_scalar_add(nrp[:], nrp[:], EPS)
            # dnp = (n2p + 1) * nrp
            nc.vector.scalar_tensor_tensor(
                dnp[:], n2p[:], 1.0, nrp[:], OP.add, OP.mult
            )
            nc.vector.reciprocal(rcp[:], dnp[:])
            nc.vector.tensor_mul(fcp[:], n2p[:], rcp[:])
            if final:
                for cc in range(2):
                    c = 2 * p + cc
                    nc.vector.tensor_scalar_mul(
                        v_sb[:, c, :], sv[:, cc, :], fcp[:, cc : cc + 1]
                    )
            else:
                with nc.allow_low_precision(reason="v to bf16 for T matmuls"):
                    for cc in range(2):
                        c = 2 * p + cc
                        nc.vector.tensor_scalar_mul(
                            vbf[:, c, :], sv[:, cc, :], fcp[:, cc : cc + 1]
                        )

        def vT_write_pair(p, slot):
            """Stage classes 2p,2p+1 of vbf in DRAM (SP-initiated)."""
            nc.sync.dma_start(vdr[slot, p], vbf[:, 2 * p : 2 * p + 2, :])

        def vT_read_pair(p, slot):
            """Read a staged pair back transposed into its block-diagonal vT
            slots. bf16: vT[32p+16cc+d, 64cc+b] = v[b,2p+cc,d]. fp8/DoubleRow:
            vT[32p+8cc+d//2, d%2, 64cc+b]. Reads spread over SP/ACT queues."""
            dst_tile = vT if p < 4 else vT4
            r0 = 32 * p if p < 4 else 0
            if FP8_T:
                engs = (nc.sync, nc.scalar)
                n = 0
                for cc in range(2):
                    for ko in range(2):
                        engs[n % 2].dma_start(
                            dst_tile[r0 + 8 * cc : r0 + 8 * cc + 8, ko,
                                     64 * cc : 64 * cc + 64],
                            vdr[slot, p, :, cc, ko::2].rearrange("b d -> d b"),
                        )
                        n += 1
            else:
                for eng, cc in ((nc.sync, 0), (nc.scalar, 1)):
                    eng.dma_start(
                        dst_tile[r0 + 16 * cc : r0 + 16 * cc + 16,
                                 64 * cc : 64 * cc + 64],
                        vdr[slot, p, :, cc, :].rearrange("b d -> d b"),
                    )

        def vT_dma_pair(p, slot):
            vT_write_pair(p, slot)
            vT_read_pair(p, slot)

        def L_front(j, p):
            """T matmuls + evac/mul for pass p; returns the P tile."""
            if p < 4:
                row0, col0 = 32 * p, 0
            else:
                row0, col0 = 0, I_CAPS
            if FP8_T:
                vrhs = (vT if p < 4 else vT4)[row0 : row0 + 16, :, :]
            else:
                vrhs = (vT if p < 4 else vT4)[row0 : row0 + 32, :]
            P = itp.tile([128, K_DIM, NG, 128], bf16, tag="pp")
            for k in range(K_DIM):
                pt = psT.tile([128, I_CAPS], f32, tag="pt")
                for g in range(NG):
                    if FP8_T:
                        nc.tensor.matmul(
                            pt[:, 128 * g : 128 * (g + 1)],
                            WT[row0 : row0 + 16, k, :,
                               col0 + 128 * g : col0 + 128 * (g + 1)],
                            vrhs,
                            start=True,
                            stop=True,
                            tile_position=(row0, 0),
                            perf_mode=PM.DoubleRow,
                        )
                    else:
                        nc.tensor.matmul(
                            pt[:, 128 * g : 128 * (g + 1)],
                            WT[row0 : row0 + 32, k,
                               col0 + 128 * g : col0 + 128 * (g + 1)],
                            vrhs,
                            start=True,
                            stop=True,
                            tile_position=(row0, 0),
                        )
                ubc = uTk[:, k].rearrange("r g b -> r g () b").to_broadcast(
                    (128, NG, 2, B_PER)
                )
                pk = P[:, k].rearrange("r g (c b) -> r g c b", c=2)
                if k in Z_KS:
                    # fused: P_k = T_k(PSUM) * u_k on DVE
                    nc.vector.tensor_tensor(
                        pk,
                        pt[:].rearrange("r (g c b) -> r g c b", g=NG, c=2),
                        ubc,
                        OP.mult,
                    )
                else:
                    Tp = itp.tile([128, NG, 128], bf16, tag="tp", bufs=6)
                    nc.scalar.copy(Tp[:].rearrange("r g q -> r (g q)"), pt[:])
                    tv = Tp[:].rearrange("r g (c b) -> r g c b", c=2)
                    # scalar_tensor_tensor (TensorScalarPtr) runs in the DVE
                    # 4x_2p perf mode on all-SBUF packed bf16 operands; plain
                    # tensor_tensor only reaches 2x.
                    if k in POOL_MUL_KS:
                        nc.gpsimd.scalar_tensor_tensor(
                            pk, tv, 1.0, ubc, OP.mult, OP.mult
                        )
                    else:
                        nc.vector.tensor_tensor(pk, tv, ubc, OP.mult)
            return P

        def L_back(j, p, P, bts, cE_prev, cE):
            """ksum + exp + cE product + denominator folds for pass p."""
            Pf = P[:].rearrange("r k g q -> r k (g q)")

            def cegv(t, g0, g1):
                # [r, g, ch, b] view over g block (enumeration order of Linc)
                return t[:, p, :, g0:g1, :].rearrange("r c g b -> r g c b")

            for g0, g1 in ((0, 4), (4, 8), (8, NG)):
                s, e = 128 * g0, 128 * g1
                lp = psL.tile([128, 512], f32, tag="lp")
                for k in range(K_DIM):
                    nc.tensor.matmul(
                        lp[:, 0 : e - s],
                        eyeb_sb[:],
                        Pf[:, k, s:e],
                        start=(k == 0),
                        stop=(k == K_DIM - 1),
                    )
                lpv = lp[:, 0 : e - s].rearrange(
                    "r (g c b) -> r g c b", c=2, b=B_PER
                )
                if j == 0:
                    nc.scalar.activation(
                        cegv(cE, g0, g1), lpv, AF.Exp,
                        scale=(1.0 / W8SCALE if FP8_T else 1.0),
                    )
                else:
                    Er = itp.tile([128, 512], bf16, tag="er", bufs=3)
                    erv = Er[:, 0 : e - s].rearrange(
                        "r (g c b) -> r g c b", c=2, b=B_PER
                    )
                    nc.scalar.activation(
                        erv, lpv, AF.Exp,
                        scale=(1.0 / W8SCALE if FP8_T else 1.0),
                    )
                    ce_eng = (
                        nc.gpsimd
                        if (CE_POOL and (g0 < 8 or not CE_ALT))
                        else nc.vector
                    )
                    if ce_eng is nc.gpsimd:
                        ce_eng.scalar_tensor_tensor(
                            cegv(cE, g0, g1), erv, 1.0, cegv(cE_prev, g0, g1),
                            OP.mult, OP.mult,
                        )
                    else:
                        ce_eng.tensor_tensor(
                            cegv(cE, g0, g1), erv, cegv(cE_prev, g0, g1),
                            OP.mult,
                        )
            # progressive denominator folds; the last pass's fold and the
            # final partial sum sit on the recT critical chain, so they go
            # to DVE even when the earlier folds run on GPSIMD
            eng = nc.gpsimd if (FOLDS_POOL and p < 3) else nc.vector

            def _fold(e, dst, a, b):
                if e is nc.gpsimd:
                    e.scalar_tensor_tensor(dst, a, 1.0, b, OP.mult, OP.add)
                else:
                    e.tensor_tensor(dst, a, b, OP.add)

            _fold(eng, bts[p][:], cE[:, p, 0], cE[:, p, 1])
            if p == 1:
                _fold(eng, bts[0][:], bts[0][:], bts[1][:])
            elif p == 3:
                _fold(eng, bts[2][:], bts[2][:], bts[3][:])
                # dtmp = (b0+b1) + (b2+b3) ready before pass 4 lands
                _fold(nc.vector, dtmp[:], bts[0][:], bts[2][:])

        def L_phase(j, bts):
            """Software-pipelined: pass p's back-end is emitted after pass
            p+1's front-end so its cross-engine waits are already satisfied
            when they reach the strict-FIFO engine queues."""
            cE_prev, cE = (None, cEa) if j == 0 else (cEa, cEb)
            Ps = [None] * 5
            for p in range(5):
                Ps[p] = L_front(j, p)
                if p >= 1:
                    L_back(j, p - 1, Ps[p - 1], bts, cE_prev, cE)
            L_back(j, 4, Ps[4], bts, cE_prev, cE)
            return cE

        def softmax_phase(bts):
            nc.vector.tensor_tensor(den[:], dtmp[:], bts[4][:], OP.add)
            with nc.allow_low_precision(reason="softmax reciprocal to bf16 ok"):
                nc.vector.reciprocal(
                    recT[:].rearrange("r g b -> r (g b)"),
                    den[:].rearrange("r g b -> r (g b)"),
                )

        def s_phase_routed(cE, pipe_vT, xc_dma):
            uTs = itp.tile([128, K_DIM, NG, B_PER], bf16, tag="uts", bufs=1)
            nc.vector.tensor_tensor(
                uTs[:],
                uTk[:],
                recT[:].rearrange("r g b -> r () g b").to_broadcast(
                    (128, K_DIM, NG, B_PER)
                ),
                OP.mult,
            )
            def xc_front(c):
                p, ch = c // 2, c % 2
                xc = itp.tile(
                    [128, K_DIM, NG, B_PER], bf16,
                    tag=("xcd" if c in xc_dma else "xc"),
                    bufs=(1 if c in xc_dma else 2),
                )
                cbc = cE[:, p, ch].rearrange("r g b -> r () g b").to_broadcast(
                    (128, K_DIM, NG, B_PER)
                )
                if c in xc_dma:
                    # xc = us (SP DMA copy), then xc *= cE_c (gpsimd DMA
                    # with CCE multiply; src broadcast over k)
                    nc.sync.dma_start(xc[:], uTs[:])
                    nc.gpsimd.dma_start(xc[:], cbc, accum_op=OP.mult)
                elif c in XC_POOL:
                    nc.gpsimd.scalar_tensor_tensor(
                        xc[:], uTs[:], 1.0, cbc, OP.mult, OP.mult
                    )
                elif XC_GPOOL > 0:
                    gs = NG - XC_GPOOL
                    cb = cE[:, p, ch].rearrange("r g b -> r () g b")
                    nc.vector.tensor_tensor(
                        xc[:, :, 0:gs],
                        uTs[:, :, 0:gs],
                        cb[:, :, 0:gs].to_broadcast((128, K_DIM, gs, B_PER)),
                        OP.mult,
                    )
                    nc.gpsimd.scalar_tensor_tensor(
                        xc[:, :, gs:],
                        uTs[:, :, gs:],
                        1.0,
                        cb[:, :, gs:].to_broadcast(
                            (128, K_DIM, XC_GPOOL, B_PER)
                        ),
                        OP.mult,
                        OP.mult,
                    )
                else:
                    nc.vector.tensor_tensor(xc[:], uTs[:], cbc, OP.mult)
                return xc

            def s_back(c, xc):
                pst = psL.tile([128, 512], f32, tag="lp")
                ps = pst[0:64, 0:D_DIM]
                n = 0
                for k in range(K_DIM):
                    for g in range(NG):
                        nc.tensor.matmul(
                            ps,
                            xc[:, k, g, :],
                            Wsk[:, k, g, c, :],
                            start=(n == 0),
                            stop=(n == K_DIM * NG - 1),
                        )
                        n += 1
                nc.scalar.copy(s_sb[:, c, :], ps)

            def pair_done(cdone):
                """Classes 2p,2p+1 are in s_sb: squash the pair; round 1
                DMAs the block into vT so the next round's T matmuls can
                start, round 2 DMAs the f32 result out early."""
                if cdone % 2 != 1:
                    return
                p = cdone // 2
                if pipe_vT:
                    squash_pair(p)
                    vT_dma_pair(p, 1)
                else:
                    squash_pair(p, final=True)
                    nc.sync.dma_start(
                        v_out[:, 2 * p : 2 * p + 2, :], v_sb[:, 2 * p : 2 * p + 2, :]
                    )

            xcs = [None] * C_CLS
            for c in range(C_CLS):
                xcs[c] = xc_front(c)
                if c >= 1:
                    s_back(c - 1, xcs[c - 1])
                    pair_done(c - 1)
            s_back(C_CLS - 1, xcs[C_CLS - 1])
            pair_done(C_CLS - 1)

        # ---------------- main flow ----------------
        kstage = int(os.environ.get("KSTAGE", "99"))
        wu_n = int(os.environ.get("KV2_WU", "8"))
        for p in range(5):
            s_phase_s0_pair(p)
            if p == 0 and wu_n:
                # low-priority gap fillers: run only when the PE would
                # otherwise idle waiting on the input DMA stream
                pe_warmup(wu_n)
            squash_pair(p)
            vT_dma_pair(p, 0)
        if kstage >= 1:
            for j in range(2):
                bts = []
                for i in range(5):
                    bti = smp.tile(
                        [128, NG, B_PER], bf16, tag=f"sm{i}", bufs=2,
                        name=f"bt{i}_{j}",
                    )
                    bts.append(bti)
                cE = L_phase(j, bts)
                if kstage == 1 + 3 * j:
                    break
                softmax_phase(bts)
                if kstage == 2 + 3 * j:
                    break
                s_phase_routed(
                    cE, pipe_vT=(j == 0), xc_dma=(XC_DMA0 if j == 0 else XC_DMA1)
                )
                if kstage == 3 + 3 * j:
                    break

        for pool in (smp, itp, psL, psT, perm):
            try:
                pool.release()
            except Exception:
                pass

    nc.compile()
    return nc


def _consts():
    import ml_dtypes

    return {"eyebf": np.eye(128, dtype=np.float32).astype(ml_dtypes.bfloat16)}


def _prep_w(W0):
    """Host-side layout marshalling of the replicated weights (pure
    permutation + bf16 cast; done once, shared by all cores)."""
    import ml_dtypes

    bf = ml_dtypes.bfloat16
    W0 = np.ascontiguousarray(W0, dtype=np.float32)  # [1152, 10, 16, 8]
    wsk = np.ascontiguousarray(
        W0.reshape(NG, 128, C_CLS, D_DIM, K_DIM).transpose(1, 4, 0, 2, 3)
    ).astype(bf)  # [128, k, g, c, d]
    if FP8_T:
        f8 = ml_dtypes.float8_e4m3
        wt = np.zeros((128, K_DIM, 2, 2 * I_CAPS), dtype=f8)
        Ws = (W0 * W8SCALE).astype(f8)
        for p in range(5):
            r0, c0 = (32 * p, 0) if p < 4 else (0, I_CAPS)
            for cc in range(2):
                for d in range(D_DIM):
                    # [i, k] -> row r0+8cc+d//2, ko=d%2
                    wt[r0 + 8 * cc + d // 2, :, d % 2, c0 : c0 + I_CAPS] = Ws[
                        :, 2 * p + cc, d, :
                    ].T
        return wsk, wt
    wt = np.zeros((128, K_DIM, 2 * I_CAPS), dtype=bf)
    wt[:, :, 0:I_CAPS] = (
        W0[:, 0:8].transpose(1, 2, 3, 0).reshape(128, K_DIM, I_CAPS).astype(bf)
    )  # rows 16c+d, classes 0-7
    wt[0:32, :, I_CAPS:] = (
        W0[:, 8:10].transpose(1, 2, 3, 0).reshape(32, K_DIM, I_CAPS).astype(bf)
    )  # rows 16(c-8)+d, classes 8,9
    return wsk, wt


def _prep_u(ush):
    import ml_dtypes

    return np.ascontiguousarray(
        ush.reshape(B_PER, NG, 128, K_DIM).transpose(2, 3, 1, 0)
    ).astype(ml_dtypes.bfloat16)  # [128, k, g, b]


def get_nc():
    if "nc" not in _CACHE:
        _CACHE["nc"] = _build()
    return _CACHE["nc"]


def make_in_maps(u, W):
    consts = _consts()
    wsk, wt = _prep_w(W[0])
    in_maps = []
    for core in range(N_CORES):
        sh = np.ascontiguousarray(
            u[core * B_PER : (core + 1) * B_PER], dtype=np.float32
        )
        in_maps.append(
            {
                "uTk_h": _prep_u(sh),
                "wsk_h": wsk,
                "wt_h": wt,
                **consts,
            }
        )
    return in_maps


def kernel(u: np.ndarray, W: np.ndarray) -> np.ndarray:
    from concourse.bass_utils import run_bass_kernel_spmd

    nc = get_nc()
    in_maps = make_in_maps(u, W)
    res = run_bass_kernel_spmd(nc, in_maps, list(range(N_CORES)))
    out = np.concatenate([res.results[i]["v"] for i in range(N_CORES)], axis=0)
    return out.astype(np.float32)

